# revision 24
# baseline (speedup 1.0000x reference)
"""Trainium2 Bass kernel: dense transformer attention block (QKV proj + RoPE +
GQA causal attention + output proj), tensor-parallel over 8 NeuronCores.

Sharding: heads are split across cores (4 Q heads + 1 KV head per core).
Each core computes its QKV shard for all tokens (bf16 matmuls, N=512 moving
operands), applies RoPE on the fly, and keeps Q/K/V entirely SBUF-resident
(V is transposed to [token, d] layout on the PE array; nothing round-trips
through DRAM).  Attention runs as a software-pipelined scores->exp->PV loop:
TensorE does the two matmuls per k-tile with PV skewed one k-tile behind
scores, ScalarE the exp, VectorE the bf16 column sums, diagonal mask, and
softmax normalization (GpSimd tensor ops measure ~2x slower than DVE, so
nothing elementwise goes there).  Causal diagonal blocks are width-restricted
so fully-masked columns are never computed.  Per-head attention outputs are
written to DRAM in bf16 and AllGathered per 256-token chunk (the
empirically fastest collective shape), and each core computes a 512-column
slice of the output projection, paced behind attention progress.
"""

from contextlib import ExitStack

import numpy as np
import ml_dtypes

import concourse.bass as bass
from concourse import bacc
import concourse.tile as tile
import concourse.mybir as mybir
from concourse.bass_utils import run_bass_kernel_spmd
from concourse.masks import make_identity

F32 = mybir.dt.float32
BF16 = mybir.dt.bfloat16
EXP = mybir.ActivationFunctionType.Exp

N_CORES = 8
N_HEADS = 32
N_KV_HEADS = 8
D = 128          # head dim
HID = 4096
B = 2
S = 2048
T = B * S        # 4096 tokens
ROPE_BASE = 10000.0

HL = N_HEADS // N_CORES          # 4 local Q heads per core
JC = HID // N_CORES              # 512 output columns per core
TC = 512                         # token chunk (projection, attention)
N_CH = T // TC                   # 8 chunks
N_HT = HID // 128                # 32 hidden k-tiles
N_QC = S // TC                   # 4 q-chunks per batch
KPQ = TC // 128                  # 4 k-tiles per q-chunk
AGC = 256                        # AllGather chunk granularity (tokens)
N_AGCH = T // AGC                # 16 AllGather chunks


def _emit(tc_ctx, xt, wqkvt, wot, ropes, out_t, ag_ins, ag_outs):
    nc = tc_ctx.nc

    with ExitStack() as es:
        const = es.enter_context(tc_ctx.tile_pool(name="const", bufs=1))
        ident = const.tile([128, 128], BF16)
        make_identity(nc, ident)
        # mask0[k, j] = 1.0 if j >= k else 0 (shared triangle for every
        # diagonal block after width restriction).
        mask0 = const.tile([128, TC], BF16)
        nc.vector.memset(mask0, 1.0)
        nc.gpsimd.affine_select(
            out=mask0,
            in_=mask0,
            compare_op=mybir.AluOpType.is_ge,
            fill=0.0,
            base=0,
            pattern=[[1, TC]],
            channel_multiplier=-1,
        )
        ones = const.tile([128, 128], BF16)
        nc.vector.memset(ones, 1.0)

        # Q/K/V live in SBUF end-to-end.
        persist = es.enter_context(tc_ctx.tile_pool(name="persist", bufs=1))
        q_sb = persist.tile([128, HL, T], BF16)        # [d, head, tok]
        k_sb = persist.tile([128, B, S], BF16)         # [d, b, tok]
        v_sb = persist.tile([128, B, S // 128, 128], BF16)  # [tok128, b, kt, d]

        # ---------------- Phase 1: QKV projection + RoPE --------------------
        with tc_ctx.tile_pool(name="p1_w", bufs=1) as wpool, \
             tc_ctx.tile_pool(name="p1_x", bufs=2) as xpool, \
             tc_ctx.tile_pool(name="p1_rope", bufs=2) as rpool, \
             tc_ctx.tile_pool(name="p1_ps", bufs=2, space="PSUM") as pspool, \
             tc_ctx.tile_pool(name="p1_vps", bufs=2, space="PSUM") as vpspool, \
             tc_ctx.tile_pool(name="p1_bf", bufs=2) as bfpool:
            wq_sb = wpool.tile([128, HL + 2, N_HT, 128], BF16)
            for ot in range(HL + 2):
                nc.scalar.dma_start(out=wq_sb[:, ot], in_=wqkvt.ap()[:, ot])
            for ch in range(N_CH):
                b = ch // N_QC
                s0 = (ch % N_QC) * TC
                t0 = ch * TC
                xt_sb = xpool.tile([128, N_HT, TC], BF16, name="xt_sb")
                for hq in range(2):
                    nc.sync.dma_start(
                        out=xt_sb[:, hq * 16:(hq + 1) * 16, :],
                        in_=xt.ap()[:, ch, hq * 16:(hq + 1) * 16, :],
                    )
                rope_sb = rpool.tile([128, 4, TC], BF16, name="rope_sb")
                nc.sync.dma_start(out=rope_sb, in_=ropes.ap()[:, ch])
                for ot in range(HL + 2):
                    ps = pspool.tile([128, TC], F32, tag="qkv", name="ps_qkv")
                    for h in range(N_HT):
                        nc.tensor.matmul(
                            ps,
                            lhsT=wq_sb[:, ot, h, :],
                            rhs=xt_sb[:, h, :],
                            start=(h == 0),
                            stop=(h == N_HT - 1),
                        )
                    if ot < HL + 1:
                        # RoPE for Q (ot<HL, with 1/sqrt(d) folded into the
                        # tables) and K (ot==HL); bf16 DVE ops throughout.
                        psb = bfpool.tile([128, TC], BF16, tag="psb",
                                          name="psb")
                        nc.scalar.copy(psb, ps)
                        ci = 0 if ot < HL else 2
                        sh = bfpool.tile([128, TC], BF16, tag="sh", name="sh")
                        nc.vector.tensor_copy(sh[0:64, :], psb[64:128, :])
                        nc.vector.tensor_copy(sh[64:128, :], psb[0:64, :])
                        nc.vector.tensor_mul(sh, sh, rope_sb[:, ci + 1, :])
                        tmp = bfpool.tile([128, TC], BF16, tag="tmp",
                                          name="tmp")
                        nc.vector.tensor_mul(tmp, psb, rope_sb[:, ci, :])
                        if ot < HL:
                            nc.vector.tensor_add(
                                q_sb[:, ot, t0:t0 + TC], tmp, sh)
                        else:
                            nc.vector.tensor_add(
                                k_sb[:, b, s0:s0 + TC], tmp, sh)
                    else:
                        # V: convert to bf16 and transpose to [tok, d] via PE.
                        vtmp = bfpool.tile([128, TC], BF16, tag="vtmp",
                                           name="vtmp")
                        nc.scalar.copy(vtmp, ps)
                        vps = vpspool.tile([128, KPQ, 128], BF16, name="vps")
                        for j in range(KPQ):
                            nc.tensor.transpose(
                                vps[:, j, :],
                                vtmp[:, j * 128:(j + 1) * 128],
                                ident,
                            )
                        kt0 = s0 // 128
                        nc.vector.tensor_copy(
                            v_sb[:, b, kt0:kt0 + KPQ, :], vps)

        # -------- Phases 2-3: attention + chunked AllGather + out-proj ------
        with tc_ctx.tile_pool(name="p2_pt", bufs=5) as ptpool, \
             tc_ctx.tile_pool(name="p2_cs", bufs=2) as cspool, \
             tc_ctx.tile_pool(name="p2_rc", bufs=2) as rcpool, \
             tc_ctx.tile_pool(name="p2_ab", bufs=3) as abpool, \
             tc_ctx.tile_pool(name="p2_wo", bufs=1) as wopool, \
             tc_ctx.tile_pool(name="p2_ps_s", bufs=3, space="PSUM") as ps_s, \
             tc_ctx.tile_pool(name="p2_ps_o", bufs=2, space="PSUM") as ps_o, \
             tc_ctx.tile_pool(name="p2_ps_b", bufs=1, space="PSUM") as ps_b, \
             tc_ctx.tile_pool(name="p3_ps", bufs=2, space="PSUM") as ps4pool, \
             tc_ctx.tile_pool(name="p3_ag", bufs=3) as agpool, \
             tc_ctx.tile_pool(name="p3_res", bufs=2) as respool:
            wo_sb = wopool.tile([128, N_HT, JC], BF16)
            nc.scalar.dma_start(out=wo_sb, in_=wot.ap())

            pace = {"inst": None}

            def emit_oproj(k):
                qc_k, cb = divmod(k, 2 * B)
                b_k, sub = divmod(cb, 2)
                t0 = b_k * S + qc_k * TC + sub * AGC
                ag_sb = agpool.tile([128, N_HT, AGC], BF16, tag="ag",
                                    name="ag_sb")
                ld = nc.sync.dma_start(
                    out=ag_sb,
                    in_=ag_outs[k].rearrange("(ht p) t -> p ht t", p=128),
                )
                if pace["inst"] is not None:
                    # Pace AllGather-output consumption behind real attention
                    # progress; the scheduler's collective latency estimate is
                    # optimistic and otherwise stalls the in-order PE stream.
                    bass._add_dep_helper(
                        ld.ins, pace["inst"], sync=True,
                        reason="oproj paced behind attention",
                    )
                for jt in range(JC // 128):
                    ps4 = ps4pool.tile([128, AGC], F32, name="ps4")
                    for h in range(N_HT):
                        nc.tensor.matmul(
                            ps4,
                            lhsT=wo_sb[:, h, jt * 128:(jt + 1) * 128],
                            rhs=ag_sb[:, h, :],
                            start=(h == 0),
                            stop=(h == N_HT - 1),
                        )
                    res = respool.tile([128, AGC], F32, name="res")
                    nc.scalar.copy(res, ps4)
                    nc.scalar.dma_start(
                        out=out_t[jt * 128:(jt + 1) * 128, t0:t0 + AGC],
                        in_=res,
                    )

            pending = []
            for qc in range(N_QC):
                for b in range(B):
                    c = qc * B + b
                    kt_max = KPQ * (qc + 1)
                    tq0 = b * S + qc * TC
                    for hh in range(HL):
                        pso = ps_o.tile([128, TC], F32, name="pso")
                        colsum = cspool.tile([128, TC], BF16, name="colsum")
                        pts = {}

                        def emit_s(kt):
                            doff = kt - KPQ * qc
                            off = 128 * doff if doff > 0 else 0
                            n = TC - off
                            ps = ps_s.tile([128, TC], F32, name="ps_s")
                            nc.tensor.matmul(
                                ps[:, off:TC],
                                lhsT=k_sb[:, b, kt * 128:(kt + 1) * 128],
                                rhs=q_sb[:, hh, tq0 + off:tq0 + TC],
                                start=True,
                                stop=True,
                            )
                            pt = ptpool.tile([128, TC], BF16, name="pt")
                            nc.scalar.activation(
                                pt[:, off:TC], ps[:, off:TC], EXP)
                            if doff >= 0:
                                nc.vector.tensor_mul(
                                    pt[:, off:TC], pt[:, off:TC],
                                    mask0[:, 0:n])
                            if kt == 0:
                                nc.vector.tensor_copy(colsum, pt)
                            else:
                                nc.vector.tensor_add(
                                    colsum[:, off:TC], colsum[:, off:TC],
                                    pt[:, off:TC])
                            pts[kt] = (pt, off)

                        def emit_p(kt):
                            pt, off = pts.pop(kt)
                            nc.tensor.matmul(
                                pso[:, off:TC],
                                lhsT=v_sb[:, b, kt, :],
                                rhs=pt[:, off:TC],
                                start=(kt == 0),
                                stop=(kt == kt_max - 1),
                            )

                        # Skew PV one k-tile behind scores so the exp latency
                        # never stalls the in-order PE queue.
                        emit_s(0)
                        for kt in range(1, kt_max):
                            emit_s(kt)
                            emit_p(kt - 1)
                        emit_p(kt_max - 1)

                        # One matmul against all-ones both sums colsum over k
                        # and broadcasts the sums to all 128 partitions.
                        sums = ps_b.tile([128, TC], F32, name="sums")
                        nc.tensor.matmul(
                            sums, lhsT=ones, rhs=colsum,
                            start=True, stop=True,
                        )
                        recip = rcpool.tile([128, TC], F32, name="recip")
                        nc.vector.reciprocal_approx_fast(recip, sums)
                        attnb = abpool.tile([128, TC], BF16, name="attnb")
                        nc.vector.tensor_mul(attnb, pso, recip)
                        for sub in range(2):
                            wr = nc.scalar.dma_start(
                                out=ag_ins[2 * c + sub][
                                    hh * 128:(hh + 1) * 128, :],
                                in_=attnb[:, sub * AGC:(sub + 1) * AGC],
                            )
                            pace["inst"] = wr.ins
                    for sub in range(2):
                        ck = 2 * c + sub
                        nc.gpsimd.collective_compute(
                            "AllGather",
                            mybir.AluOpType.bypass,
                            replica_groups=[list(range(N_CORES))],
                            ins=[ag_ins[ck][:]],
                            outs=[ag_outs[ck][:]],
                        )
                        pending.append(ck)
                    # Deep lag while the AG pipeline spins up (the first
                    # chunks are small and finish before their collectives),
                    # shallow at the end so the tail drains promptly.
                    lag = 6 if c < 3 else (5 if c < 5 else (4 if c < 7 else 2))
                    while len(pending) > lag:
                        emit_oproj(pending.pop(0))
            for c in pending:
                emit_oproj(c)


def _build_program():
    nc = bacc.Bacc("TRN2", target_bir_lowering=False, debug=False,
                   num_devices=N_CORES)
    xt = nc.declare_dram_parameter("xt", [128, N_CH, N_HT, TC], BF16,
                                   isOutput=False)
    wqkvt = nc.declare_dram_parameter("wqkvt", [128, HL + 2, N_HT, 128], BF16,
                                      isOutput=False)
    wot = nc.declare_dram_parameter("wot", [128, N_HT, JC], BF16,
                                    isOutput=False)
    ropes = nc.declare_dram_parameter("ropes", [128, N_CH, 4, TC], BF16,
                                      isOutput=False)
    out_t = nc.declare_dram_parameter("out_t", [JC, T], F32, isOutput=True)

    ag_ins = [nc.dram_tensor(f"ag_in{k}", [HL * D, AGC], BF16).ap()
              for k in range(N_AGCH)]
    ag_outs = [nc.dram_tensor(f"ag_out{k}", [N_HEADS * D, AGC], BF16,
                              addr_space="Shared").ap()
               for k in range(N_AGCH)]

    with tile.TileContext(nc) as tc_ctx:
        _emit(tc_ctx, xt, wqkvt, wot, ropes, out_t, ag_ins, ag_outs)
    nc.finalize()
    return nc


def _host_inputs(hidden_states, w_qkv, w_o):
    """Shard + transpose inputs for the 8 cores; returns in_maps."""
    X = np.asarray(hidden_states, dtype=np.float32).reshape(T, HID)
    xt = np.ascontiguousarray(
        X.reshape(N_CH, TC, N_HT, 128).transpose(3, 0, 2, 1)
    ).astype(ml_dtypes.bfloat16)

    # RoPE tables in [d, t] layout with rotate-half sign folded into sin and
    # the attention scale folded into the Q tables.
    inv_freq = 1.0 / (ROPE_BASE ** (np.arange(0, D, 2, dtype=np.float32) / D))
    pos = np.arange(S, dtype=np.float32)
    freqs = np.outer(pos, inv_freq)                      # (S, D/2)
    emb = np.concatenate([freqs, freqs], axis=-1)        # (S, D)
    cos = np.cos(emb).T.astype(np.float32)               # (D, S)
    sin = np.sin(emb).T.astype(np.float32)
    sgn = np.concatenate([-np.ones(D // 2), np.ones(D // 2)]).astype(np.float32)
    sins = sgn[:, None] * sin
    cos_t = np.tile(cos, (1, B))                         # (D, T)
    sins_t = np.tile(sins, (1, B))
    scale = np.float32(D ** -0.5)
    ropes = np.stack([cos_t * scale, sins_t * scale, cos_t, sins_t], axis=0)
    ropes = np.ascontiguousarray(
        ropes.reshape(4, 128, N_CH, TC).transpose(1, 2, 0, 3)
    ).astype(ml_dtypes.bfloat16)

    w_qkv = np.asarray(w_qkv, dtype=np.float32)
    w_o = np.asarray(w_o, dtype=np.float32)
    q_sz = N_HEADS * D
    kv_sz = N_KV_HEADS * D
    in_maps = []
    for c in range(N_CORES):
        qr = w_qkv[c * HL * D:(c + 1) * HL * D]
        kr = w_qkv[q_sz + c * D:q_sz + (c + 1) * D]
        vr = w_qkv[q_sz + kv_sz + c * D:q_sz + kv_sz + (c + 1) * D]
        w_shard = np.concatenate([qr, kr, vr], axis=0)           # (768, HID)
        wqkvt_c = np.ascontiguousarray(
            w_shard.reshape(HL + 2, 128, N_HT, 128).transpose(3, 0, 2, 1)
        ).astype(ml_dtypes.bfloat16)
        wot_c = np.ascontiguousarray(
            w_o[c * JC:(c + 1) * JC, :].reshape(JC, N_HT, 128).transpose(2, 1, 0)
        ).astype(ml_dtypes.bfloat16)
        in_maps.append({
            "xt": xt, "wqkvt": wqkvt_c, "wot": wot_c, "ropes": ropes,
        })
    return in_maps


def _run(hidden_states, w_qkv, w_o, trace=False, tmpdir=None):
    in_maps = _host_inputs(hidden_states, w_qkv, w_o)
    nc = _build_program()
    res = run_bass_kernel_spmd(nc, in_maps, list(range(N_CORES)),
                               trace=trace, tmpdir=tmpdir)
    out_T = np.concatenate(
        [np.asarray(res.results[c]["out_t"]) for c in range(N_CORES)], axis=0
    )                                                     # (HID j, T)
    out = np.ascontiguousarray(out_T.T).reshape(B, S, HID).astype(np.float32)
    return out, res


def kernel(hidden_states, w_qkv, w_o):
    out, _ = _run(hidden_states, w_qkv, w_o, trace=False)
    return out


# revision 28
# speedup vs baseline: 1.0754x; 1.0754x over previous
"""Trainium2 Bass kernel: dense transformer attention block (QKV proj + RoPE +
GQA causal attention + output proj), tensor-parallel over 8 NeuronCores.

Sharding: heads are split across cores (4 Q heads + 1 KV head per core).
Each core computes its QKV shard for all tokens (bf16 matmuls, N=512 moving
operands), applies RoPE on the fly, and keeps Q/K/V entirely SBUF-resident
(V is transposed to [token, d] layout on the PE array; nothing round-trips
through DRAM).  Attention runs as a software-pipelined scores->exp->PV loop:
TensorE does the two matmuls per k-tile with PV skewed one k-tile behind
scores, ScalarE the exp, VectorE the bf16 column sums, diagonal mask, and
softmax normalization (GpSimd tensor ops measure ~2x slower than DVE, so
nothing elementwise goes there).  Causal diagonal blocks are width-restricted
so fully-masked columns are never computed.  Per-head attention outputs are
written to DRAM in bf16 and AllGathered per 256-token chunk (the
empirically fastest collective shape), and each core computes a 512-column
slice of the output projection, paced behind attention progress.
"""

from contextlib import ExitStack

import numpy as np
import ml_dtypes

import concourse.bass as bass
from concourse import bacc
import concourse.tile as tile
import concourse.mybir as mybir
from concourse.bass_utils import run_bass_kernel_spmd
from concourse.masks import make_identity

F32 = mybir.dt.float32
BF16 = mybir.dt.bfloat16
EXP = mybir.ActivationFunctionType.Exp

N_CORES = 8
N_HEADS = 32
N_KV_HEADS = 8
D = 128          # head dim
HID = 4096
B = 2
S = 2048
T = B * S        # 4096 tokens
ROPE_BASE = 10000.0

HL = N_HEADS // N_CORES          # 4 local Q heads per core
JC = HID // N_CORES              # 512 output columns per core
TC = 512                         # token chunk (projection, attention)
N_CH = T // TC                   # 8 chunks
N_HT = HID // 128                # 32 hidden k-tiles
N_QC = S // TC                   # 4 q-chunks per batch
KPQ = TC // 128                  # 4 k-tiles per q-chunk
AGC = 256                        # AllGather chunk granularity (tokens)
N_AGCH = T // AGC                # 16 AllGather chunks


def _emit(tc_ctx, xt, wqkvt, wot, ropes, out_t, ag_ins, ag_outs):
    nc = tc_ctx.nc

    with ExitStack() as es:
        const = es.enter_context(tc_ctx.tile_pool(name="const", bufs=1))
        ident = const.tile([128, 128], BF16)
        make_identity(nc, ident)
        # mask0[k, j] = 1.0 if j >= k else 0 (shared triangle for every
        # diagonal block after width restriction).
        mask0 = const.tile([128, TC], BF16)
        nc.vector.memset(mask0, 1.0)
        nc.gpsimd.affine_select(
            out=mask0,
            in_=mask0,
            compare_op=mybir.AluOpType.is_ge,
            fill=0.0,
            base=0,
            pattern=[[1, TC]],
            channel_multiplier=-1,
        )
        ones = const.tile([128, 128], BF16)
        nc.vector.memset(ones, 1.0)

        # Q/K/V live in SBUF end-to-end.
        persist = es.enter_context(tc_ctx.tile_pool(name="persist", bufs=1))
        q_sb = persist.tile([128, HL, T], BF16)        # [d, head, tok]
        k_sb = persist.tile([128, B, S], BF16)         # [d, b, tok]
        v_sb = persist.tile([128, B, S // 128, 128], BF16)  # [tok128, b, kt, d]

        # Attention pools live for the whole kernel: batch-0 attention (and
        # its AllGathers) interleaves into phase 1 so the serialized
        # collective engine starts ~250us earlier and the back half consumes
        # pre-gathered chunks.  The aux PSUM bank is shared between the
        # phase-1 V-transpose tiles and the softmax-sum tiles (both
        # low-duty) to stay within 8 banks.
        ptpool = es.enter_context(tc_ctx.tile_pool(name="p2_pt", bufs=5))
        cspool = es.enter_context(tc_ctx.tile_pool(name="p2_cs", bufs=2))
        rcpool = es.enter_context(tc_ctx.tile_pool(name="p2_rc", bufs=2))
        abpool = es.enter_context(tc_ctx.tile_pool(name="p2_ab", bufs=3))
        ps_s = es.enter_context(
            tc_ctx.tile_pool(name="p2_ps_s", bufs=3, space="PSUM"))
        ps_o = es.enter_context(
            tc_ctx.tile_pool(name="p2_ps_o", bufs=2, space="PSUM"))
        aux_ps = es.enter_context(
            tc_ctx.tile_pool(name="p2_aux", bufs=1, space="PSUM"))

        pace = {"inst": None}
        pending = []

        def emit_attn_chunk(qc, b):
            c = qc * B + b
            kt_max = KPQ * (qc + 1)
            tq0 = b * S + qc * TC
            for hh in range(HL):
                pso = ps_o.tile([128, TC], F32, name="pso")
                colsum = cspool.tile([128, TC], BF16, name="colsum")
                pts = {}

                def emit_s(kt):
                    doff = kt - KPQ * qc
                    off = 128 * doff if doff > 0 else 0
                    n = TC - off
                    ps = ps_s.tile([128, TC], F32, name="ps_s")
                    nc.tensor.matmul(
                        ps[:, off:TC],
                        lhsT=k_sb[:, b, kt * 128:(kt + 1) * 128],
                        rhs=q_sb[:, hh, tq0 + off:tq0 + TC],
                        start=True,
                        stop=True,
                    )
                    pt = ptpool.tile([128, TC], BF16, name="pt")
                    nc.scalar.activation(pt[:, off:TC], ps[:, off:TC], EXP)
                    if doff >= 0:
                        nc.vector.tensor_mul(
                            pt[:, off:TC], pt[:, off:TC], mask0[:, 0:n])
                    if kt == 0:
                        nc.vector.tensor_copy(colsum, pt)
                    else:
                        nc.vector.tensor_add(
                            colsum[:, off:TC], colsum[:, off:TC],
                            pt[:, off:TC])
                    pts[kt] = (pt, off)

                def emit_p(kt):
                    pt, off = pts.pop(kt)
                    nc.tensor.matmul(
                        pso[:, off:TC],
                        lhsT=v_sb[:, b, kt, :],
                        rhs=pt[:, off:TC],
                        start=(kt == 0),
                        stop=(kt == kt_max - 1),
                    )

                # Skew PV one k-tile behind scores so the exp latency
                # never stalls the in-order PE queue.
                emit_s(0)
                for kt in range(1, kt_max):
                    emit_s(kt)
                    emit_p(kt - 1)
                emit_p(kt_max - 1)

                # One matmul against all-ones both sums colsum over k
                # and broadcasts the sums to all 128 partitions.
                sums = aux_ps.tile([128, TC], F32, tag="aux", name="sums")
                nc.tensor.matmul(
                    sums, lhsT=ones, rhs=colsum, start=True, stop=True)
                recip = rcpool.tile([128, TC], F32, name="recip")
                nc.vector.reciprocal_approx_fast(recip, sums)
                attnb = abpool.tile([128, TC], BF16, name="attnb")
                nc.vector.tensor_mul(attnb, pso, recip)
                for sub in range(2):
                    wr = nc.scalar.dma_start(
                        out=ag_ins[2 * c + sub][hh * 128:(hh + 1) * 128, :],
                        in_=attnb[:, sub * AGC:(sub + 1) * AGC],
                    )
                    pace["inst"] = wr.ins
            for sub in range(2):
                ck = 2 * c + sub
                nc.gpsimd.collective_compute(
                    "AllGather",
                    mybir.AluOpType.bypass,
                    replica_groups=[list(range(N_CORES))],
                    ins=[ag_ins[ck][:]],
                    outs=[ag_outs[ck][:]],
                )
                pending.append(ck)

        # ---------------- Phase 1: QKV projection + RoPE --------------------
        with tc_ctx.tile_pool(name="p1_w", bufs=1) as wpool, \
             tc_ctx.tile_pool(name="p1_x", bufs=2) as xpool, \
             tc_ctx.tile_pool(name="p1_rope", bufs=2) as rpool, \
             tc_ctx.tile_pool(name="p1_ps", bufs=2, space="PSUM") as pspool, \
             tc_ctx.tile_pool(name="p1_bf", bufs=2) as bfpool:
            wq_sb = wpool.tile([128, HL + 2, N_HT, 128], BF16)
            for ot in range(HL + 2):
                nc.scalar.dma_start(out=wq_sb[:, ot], in_=wqkvt.ap()[:, ot])
            for ch in range(N_CH):
                b = ch // N_QC
                s0 = (ch % N_QC) * TC
                t0 = ch * TC
                xt_sb = xpool.tile([128, N_HT, TC], BF16, name="xt_sb")
                for hq in range(2):
                    nc.sync.dma_start(
                        out=xt_sb[:, hq * 16:(hq + 1) * 16, :],
                        in_=xt.ap()[:, ch, hq * 16:(hq + 1) * 16, :],
                    )
                rope_sb = rpool.tile([128, 4, TC], BF16, name="rope_sb")
                nc.sync.dma_start(out=rope_sb, in_=ropes.ap()[:, ch])
                for ot in range(HL + 2):
                    ps = pspool.tile([128, TC], F32, tag="qkv", name="ps_qkv")
                    for h in range(N_HT):
                        nc.tensor.matmul(
                            ps,
                            lhsT=wq_sb[:, ot, h, :],
                            rhs=xt_sb[:, h, :],
                            start=(h == 0),
                            stop=(h == N_HT - 1),
                        )
                    if ot < HL + 1:
                        # RoPE for Q (ot<HL, with 1/sqrt(d) folded into the
                        # tables) and K (ot==HL); bf16 DVE ops throughout.
                        psb = bfpool.tile([128, TC], BF16, tag="psb",
                                          name="psb")
                        nc.scalar.copy(psb, ps)
                        ci = 0 if ot < HL else 2
                        sh = bfpool.tile([128, TC], BF16, tag="sh", name="sh")
                        nc.vector.tensor_copy(sh[0:64, :], psb[64:128, :])
                        nc.vector.tensor_copy(sh[64:128, :], psb[0:64, :])
                        nc.vector.tensor_mul(sh, sh, rope_sb[:, ci + 1, :])
                        tmp = bfpool.tile([128, TC], BF16, tag="tmp",
                                          name="tmp")
                        nc.vector.tensor_mul(tmp, psb, rope_sb[:, ci, :])
                        if ot < HL:
                            nc.vector.tensor_add(
                                q_sb[:, ot, t0:t0 + TC], tmp, sh)
                        else:
                            nc.vector.tensor_add(
                                k_sb[:, b, s0:s0 + TC], tmp, sh)
                    else:
                        # V: convert to bf16 and transpose to [tok, d] via PE.
                        vtmp = bfpool.tile([128, TC], BF16, tag="vtmp",
                                           name="vtmp")
                        nc.scalar.copy(vtmp, ps)
                        vps = aux_ps.tile([128, KPQ, 128], BF16, tag="aux",
                                          name="vps")
                        for j in range(KPQ):
                            nc.tensor.transpose(
                                vps[:, j, :],
                                vtmp[:, j * 128:(j + 1) * 128],
                                ident,
                            )
                        kt0 = s0 // 128
                        nc.vector.tensor_copy(
                            v_sb[:, b, kt0:kt0 + KPQ, :], vps)
                if ch < N_QC:
                    # Batch-0 q/k/v rows through q-chunk `ch` are complete:
                    # run its attention (and fire its AllGathers) now so the
                    # collective pipeline spins up during phase 1.
                    emit_attn_chunk(ch, 0)

        # -------- Back half: batch-1 attention + out-proj drain -------------
        with tc_ctx.tile_pool(name="p2_wo", bufs=1) as wopool, \
             tc_ctx.tile_pool(name="p3_ps", bufs=2, space="PSUM") as ps4pool, \
             tc_ctx.tile_pool(name="p3_ag", bufs=3) as agpool, \
             tc_ctx.tile_pool(name="p3_res", bufs=2) as respool:
            wo_sb = wopool.tile([128, N_HT, JC], BF16)
            nc.scalar.dma_start(out=wo_sb, in_=wot.ap())

            def emit_oproj(k):
                qc_k, cb = divmod(k, 2 * B)
                b_k, sub = divmod(cb, 2)
                t0 = b_k * S + qc_k * TC + sub * AGC
                ag_sb = agpool.tile([128, N_HT, AGC], BF16, tag="ag",
                                    name="ag_sb")
                ld = nc.sync.dma_start(
                    out=ag_sb,
                    in_=ag_outs[k].rearrange("(ht p) t -> p ht t", p=128),
                )
                if pace["inst"] is not None:
                    # Pace AllGather-output consumption behind real attention
                    # progress; the scheduler's collective latency estimate is
                    # optimistic and otherwise stalls the in-order PE stream.
                    bass._add_dep_helper(
                        ld.ins, pace["inst"], sync=True,
                        reason="oproj paced behind attention",
                    )
                for jt in range(JC // 128):
                    ps4 = ps4pool.tile([128, AGC], F32, name="ps4")
                    for h in range(N_HT):
                        nc.tensor.matmul(
                            ps4,
                            lhsT=wo_sb[:, h, jt * 128:(jt + 1) * 128],
                            rhs=ag_sb[:, h, :],
                            start=(h == 0),
                            stop=(h == N_HT - 1),
                        )
                    res = respool.tile([128, AGC], F32, name="res")
                    nc.scalar.copy(res, ps4)
                    nc.scalar.dma_start(
                        out=out_t[jt * 128:(jt + 1) * 128, t0:t0 + AGC],
                        in_=res,
                    )

            for qc in range(N_QC):
                emit_attn_chunk(qc, 1)
                # Drain three pre-gathered o_proj chunks per batch-1
                # attention chunk; the last four (two ready + the final two
                # AllGathers) drain after the loop.
                while len(pending) > 7 - qc:
                    emit_oproj(pending.pop(0))
            for k in pending:
                emit_oproj(k)


def _build_program():
    nc = bacc.Bacc("TRN2", target_bir_lowering=False, debug=False,
                   num_devices=N_CORES)
    xt = nc.declare_dram_parameter("xt", [128, N_CH, N_HT, TC], BF16,
                                   isOutput=False)
    wqkvt = nc.declare_dram_parameter("wqkvt", [128, HL + 2, N_HT, 128], BF16,
                                      isOutput=False)
    wot = nc.declare_dram_parameter("wot", [128, N_HT, JC], BF16,
                                    isOutput=False)
    ropes = nc.declare_dram_parameter("ropes", [128, N_CH, 4, TC], BF16,
                                      isOutput=False)
    out_t = nc.declare_dram_parameter("out_t", [JC, T], F32, isOutput=True)

    ag_ins = [nc.dram_tensor(f"ag_in{k}", [HL * D, AGC], BF16).ap()
              for k in range(N_AGCH)]
    ag_outs = [nc.dram_tensor(f"ag_out{k}", [N_HEADS * D, AGC], BF16,
                              addr_space="Shared").ap()
               for k in range(N_AGCH)]

    with tile.TileContext(nc) as tc_ctx:
        _emit(tc_ctx, xt, wqkvt, wot, ropes, out_t, ag_ins, ag_outs)
    nc.finalize()
    return nc


def _host_inputs(hidden_states, w_qkv, w_o):
    """Shard + transpose inputs for the 8 cores; returns in_maps."""
    X = np.asarray(hidden_states, dtype=np.float32).reshape(T, HID)
    xt = np.ascontiguousarray(
        X.reshape(N_CH, TC, N_HT, 128).transpose(3, 0, 2, 1)
    ).astype(ml_dtypes.bfloat16)

    # RoPE tables in [d, t] layout with rotate-half sign folded into sin and
    # the attention scale folded into the Q tables.
    inv_freq = 1.0 / (ROPE_BASE ** (np.arange(0, D, 2, dtype=np.float32) / D))
    pos = np.arange(S, dtype=np.float32)
    freqs = np.outer(pos, inv_freq)                      # (S, D/2)
    emb = np.concatenate([freqs, freqs], axis=-1)        # (S, D)
    cos = np.cos(emb).T.astype(np.float32)               # (D, S)
    sin = np.sin(emb).T.astype(np.float32)
    sgn = np.concatenate([-np.ones(D // 2), np.ones(D // 2)]).astype(np.float32)
    sins = sgn[:, None] * sin
    cos_t = np.tile(cos, (1, B))                         # (D, T)
    sins_t = np.tile(sins, (1, B))
    scale = np.float32(D ** -0.5)
    ropes = np.stack([cos_t * scale, sins_t * scale, cos_t, sins_t], axis=0)
    ropes = np.ascontiguousarray(
        ropes.reshape(4, 128, N_CH, TC).transpose(1, 2, 0, 3)
    ).astype(ml_dtypes.bfloat16)

    w_qkv = np.asarray(w_qkv, dtype=np.float32)
    w_o = np.asarray(w_o, dtype=np.float32)
    q_sz = N_HEADS * D
    kv_sz = N_KV_HEADS * D
    in_maps = []
    for c in range(N_CORES):
        qr = w_qkv[c * HL * D:(c + 1) * HL * D]
        kr = w_qkv[q_sz + c * D:q_sz + (c + 1) * D]
        vr = w_qkv[q_sz + kv_sz + c * D:q_sz + kv_sz + (c + 1) * D]
        w_shard = np.concatenate([qr, kr, vr], axis=0)           # (768, HID)
        wqkvt_c = np.ascontiguousarray(
            w_shard.reshape(HL + 2, 128, N_HT, 128).transpose(3, 0, 2, 1)
        ).astype(ml_dtypes.bfloat16)
        wot_c = np.ascontiguousarray(
            w_o[c * JC:(c + 1) * JC, :].reshape(JC, N_HT, 128).transpose(2, 1, 0)
        ).astype(ml_dtypes.bfloat16)
        in_maps.append({
            "xt": xt, "wqkvt": wqkvt_c, "wot": wot_c, "ropes": ropes,
        })
    return in_maps


def _run(hidden_states, w_qkv, w_o, trace=False, tmpdir=None):
    in_maps = _host_inputs(hidden_states, w_qkv, w_o)
    nc = _build_program()
    res = run_bass_kernel_spmd(nc, in_maps, list(range(N_CORES)),
                               trace=trace, tmpdir=tmpdir)
    out_T = np.concatenate(
        [np.asarray(res.results[c]["out_t"]) for c in range(N_CORES)], axis=0
    )                                                     # (HID j, T)
    out = np.ascontiguousarray(out_T.T).reshape(B, S, HID).astype(np.float32)
    return out, res


def kernel(hidden_states, w_qkv, w_o):
    out, _ = _run(hidden_states, w_qkv, w_o, trace=False)
    return out


# revision 29
# speedup vs baseline: 1.1111x; 1.0333x over previous
"""Trainium2 Bass kernel: dense transformer attention block (QKV proj + RoPE +
GQA causal attention + output proj), tensor-parallel over 8 NeuronCores.

Sharding: heads are split across cores (4 Q heads + 1 KV head per core).
Each core computes its QKV shard for all tokens (bf16 matmuls, N=512 moving
operands), applies RoPE on the fly, and keeps Q/K/V entirely SBUF-resident
(V is transposed to [token, d] layout on the PE array; nothing round-trips
through DRAM).  Attention runs as a software-pipelined scores->exp->PV loop:
TensorE does the two matmuls per k-tile with PV skewed one k-tile behind
scores, ScalarE the exp, VectorE the bf16 column sums, diagonal mask, and
softmax normalization (GpSimd tensor ops measure ~2x slower than DVE, so
nothing elementwise goes there).  Causal diagonal blocks are width-restricted
so fully-masked columns are never computed.  Per-head attention outputs are
written to DRAM in bf16 and AllGathered per 256-token chunk (the
empirically fastest collective shape), and each core computes a 512-column
slice of the output projection, paced behind attention progress.
"""

from contextlib import ExitStack

import numpy as np
import ml_dtypes

import concourse.bass as bass
from concourse import bacc
import concourse.tile as tile
import concourse.mybir as mybir
from concourse.bass_utils import run_bass_kernel_spmd
from concourse.masks import make_identity

F32 = mybir.dt.float32
BF16 = mybir.dt.bfloat16
EXP = mybir.ActivationFunctionType.Exp

N_CORES = 8
N_HEADS = 32
N_KV_HEADS = 8
D = 128          # head dim
HID = 4096
B = 2
S = 2048
T = B * S        # 4096 tokens
ROPE_BASE = 10000.0

HL = N_HEADS // N_CORES          # 4 local Q heads per core
JC = HID // N_CORES              # 512 output columns per core
TC = 512                         # token chunk (projection, attention)
N_CH = T // TC                   # 8 chunks
N_HT = HID // 128                # 32 hidden k-tiles
N_QC = S // TC                   # 4 q-chunks per batch
KPQ = TC // 128                  # 4 k-tiles per q-chunk
AGC = 256                        # AllGather chunk granularity (tokens)
N_AGCH = T // AGC                # 16 AllGather chunks


def _emit(tc_ctx, xt, wqkvt, wot, ropes, out_t, ag_ins, ag_outs):
    nc = tc_ctx.nc

    with ExitStack() as es:
        const = es.enter_context(tc_ctx.tile_pool(name="const", bufs=1))
        ident = const.tile([128, 128], BF16)
        make_identity(nc, ident)
        # mask0[k, j] = 1.0 if j >= k else 0 (shared triangle for every
        # diagonal block after width restriction).
        mask0 = const.tile([128, TC], BF16)
        nc.vector.memset(mask0, 1.0)
        nc.gpsimd.affine_select(
            out=mask0,
            in_=mask0,
            compare_op=mybir.AluOpType.is_ge,
            fill=0.0,
            base=0,
            pattern=[[1, TC]],
            channel_multiplier=-1,
        )
        ones = const.tile([128, 128], BF16)
        nc.vector.memset(ones, 1.0)

        # Q/K/V live in SBUF end-to-end.
        persist = es.enter_context(tc_ctx.tile_pool(name="persist", bufs=1))
        q_sb = persist.tile([128, HL, T], BF16)        # [d, head, tok]
        k_sb = persist.tile([128, B, S], BF16)         # [d, b, tok]
        v_sb = persist.tile([128, B, S // 128, 128], BF16)  # [tok128, b, kt, d]

        # Attention pools live for the whole kernel: batch-0 attention (and
        # its AllGathers) interleaves into phase 1 so the serialized
        # collective engine starts ~250us earlier and the back half consumes
        # pre-gathered chunks.  The aux PSUM bank is shared between the
        # phase-1 V-transpose tiles and the softmax-sum tiles (both
        # low-duty) to stay within 8 banks.
        ptpool = es.enter_context(tc_ctx.tile_pool(name="p2_pt", bufs=5))
        cspool = es.enter_context(tc_ctx.tile_pool(name="p2_cs", bufs=2))
        rcpool = es.enter_context(tc_ctx.tile_pool(name="p2_rc", bufs=2))
        abpool = es.enter_context(tc_ctx.tile_pool(name="p2_ab", bufs=3))
        ps_s = es.enter_context(
            tc_ctx.tile_pool(name="p2_ps_s", bufs=3, space="PSUM"))
        ps_o = es.enter_context(
            tc_ctx.tile_pool(name="p2_ps_o", bufs=2, space="PSUM"))
        aux_ps = es.enter_context(
            tc_ctx.tile_pool(name="p2_aux", bufs=1, space="PSUM"))

        pace = {"inst": None}
        pending = []

        def emit_attn_chunk(qc, b):
            c = qc * B + b
            kt_max = KPQ * (qc + 1)
            tq0 = b * S + qc * TC
            for hh in range(HL):
                pso = ps_o.tile([128, TC], F32, name="pso")
                colsum = cspool.tile([128, TC], BF16, name="colsum")
                pts = {}

                def emit_s(kt):
                    doff = kt - KPQ * qc
                    off = 128 * doff if doff > 0 else 0
                    n = TC - off
                    ps = ps_s.tile([128, TC], F32, name="ps_s")
                    nc.tensor.matmul(
                        ps[:, off:TC],
                        lhsT=k_sb[:, b, kt * 128:(kt + 1) * 128],
                        rhs=q_sb[:, hh, tq0 + off:tq0 + TC],
                        start=True,
                        stop=True,
                    )
                    pt = ptpool.tile([128, TC], BF16, name="pt")
                    nc.scalar.activation(pt[:, off:TC], ps[:, off:TC], EXP)
                    if doff >= 0:
                        nc.vector.tensor_mul(
                            pt[:, off:TC], pt[:, off:TC], mask0[:, 0:n])
                    if kt == 0:
                        nc.vector.tensor_copy(colsum, pt)
                    else:
                        nc.vector.tensor_add(
                            colsum[:, off:TC], colsum[:, off:TC],
                            pt[:, off:TC])
                    pts[kt] = (pt, off)

                def emit_p(kt):
                    pt, off = pts.pop(kt)
                    nc.tensor.matmul(
                        pso[:, off:TC],
                        lhsT=v_sb[:, b, kt, :],
                        rhs=pt[:, off:TC],
                        start=(kt == 0),
                        stop=(kt == kt_max - 1),
                    )

                # Skew PV one k-tile behind scores so the exp latency
                # never stalls the in-order PE queue.
                emit_s(0)
                for kt in range(1, kt_max):
                    emit_s(kt)
                    emit_p(kt - 1)
                emit_p(kt_max - 1)

                # One matmul against all-ones both sums colsum over k
                # and broadcasts the sums to all 128 partitions.
                sums = aux_ps.tile([128, TC], F32, tag="aux", name="sums")
                nc.tensor.matmul(
                    sums, lhsT=ones, rhs=colsum, start=True, stop=True)
                recip = rcpool.tile([128, TC], F32, name="recip")
                nc.vector.reciprocal_approx_fast(recip, sums)
                attnb = abpool.tile([128, TC], BF16, name="attnb")
                nc.vector.tensor_mul(attnb, pso, recip)
                for sub in range(2):
                    wr = nc.scalar.dma_start(
                        out=ag_ins[2 * c + sub][hh * 128:(hh + 1) * 128, :],
                        in_=attnb[:, sub * AGC:(sub + 1) * AGC],
                    )
                    pace["inst"] = wr.ins
            for sub in range(2):
                ck = 2 * c + sub
                nc.gpsimd.collective_compute(
                    "AllGather",
                    mybir.AluOpType.bypass,
                    replica_groups=[list(range(N_CORES))],
                    ins=[ag_ins[ck][:]],
                    outs=[ag_outs[ck][:]],
                )
                pending.append(ck)

        # ---------------- Phase 1: QKV projection + RoPE --------------------
        with tc_ctx.tile_pool(name="p1_w", bufs=1) as wpool, \
             tc_ctx.tile_pool(name="p1_x", bufs=2) as xpool, \
             tc_ctx.tile_pool(name="p1_rope", bufs=2) as rpool, \
             tc_ctx.tile_pool(name="p1_ps", bufs=2, space="PSUM") as pspool, \
             tc_ctx.tile_pool(name="p1_bf", bufs=2) as bfpool:
            wq_sb = wpool.tile([128, HL + 2, N_HT, 128], BF16)
            for ot in range(HL + 2):
                nc.scalar.dma_start(out=wq_sb[:, ot], in_=wqkvt.ap()[:, ot])
            for ch in range(N_CH):
                b = ch // N_QC
                s0 = (ch % N_QC) * TC
                t0 = ch * TC
                xt_sb = xpool.tile([128, N_HT, TC], BF16, name="xt_sb")
                for hq in range(2):
                    nc.sync.dma_start(
                        out=xt_sb[:, hq * 16:(hq + 1) * 16, :],
                        in_=xt.ap()[:, ch, hq * 16:(hq + 1) * 16, :],
                    )
                rope_sb = rpool.tile([128, 4, TC], BF16, name="rope_sb")
                nc.sync.dma_start(out=rope_sb, in_=ropes.ap()[:, ch])
                for ot in range(HL + 2):
                    ps = pspool.tile([128, TC], F32, tag="qkv", name="ps_qkv")
                    for h in range(N_HT):
                        nc.tensor.matmul(
                            ps,
                            lhsT=wq_sb[:, ot, h, :],
                            rhs=xt_sb[:, h, :],
                            start=(h == 0),
                            stop=(h == N_HT - 1),
                        )
                    if ot < HL + 1:
                        # RoPE for Q (ot<HL, with 1/sqrt(d) folded into the
                        # tables) and K (ot==HL); bf16 DVE ops throughout.
                        psb = bfpool.tile([128, TC], BF16, tag="psb",
                                          name="psb")
                        nc.scalar.copy(psb, ps)
                        ci = 0 if ot < HL else 2
                        sh = bfpool.tile([128, TC], BF16, tag="sh", name="sh")
                        nc.vector.tensor_copy(sh[0:64, :], psb[64:128, :])
                        nc.vector.tensor_copy(sh[64:128, :], psb[0:64, :])
                        nc.vector.tensor_mul(sh, sh, rope_sb[:, ci + 1, :])
                        tmp = bfpool.tile([128, TC], BF16, tag="tmp",
                                          name="tmp")
                        nc.vector.tensor_mul(tmp, psb, rope_sb[:, ci, :])
                        if ot < HL:
                            nc.vector.tensor_add(
                                q_sb[:, ot, t0:t0 + TC], tmp, sh)
                        else:
                            nc.vector.tensor_add(
                                k_sb[:, b, s0:s0 + TC], tmp, sh)
                    else:
                        # V: convert to bf16 and transpose to [tok, d] via PE.
                        vtmp = bfpool.tile([128, TC], BF16, tag="vtmp",
                                           name="vtmp")
                        nc.scalar.copy(vtmp, ps)
                        vps = aux_ps.tile([128, KPQ, 128], BF16, tag="aux",
                                          name="vps")
                        for j in range(KPQ):
                            nc.tensor.transpose(
                                vps[:, j, :],
                                vtmp[:, j * 128:(j + 1) * 128],
                                ident,
                            )
                        kt0 = s0 // 128
                        nc.vector.tensor_copy(
                            v_sb[:, b, kt0:kt0 + KPQ, :], vps)
                # q/k/v rows for this batch through q-chunk `ch % N_QC`
                # are complete: run its attention (and fire its AllGathers)
                # now, so the collective pipeline runs during phase 1 and
                # the back half is a pure o_proj drain with no AG tail.
                emit_attn_chunk(ch % N_QC, ch // N_QC)

        # -------- Back half: batch-1 attention + out-proj drain -------------
        with tc_ctx.tile_pool(name="p2_wo", bufs=1) as wopool, \
             tc_ctx.tile_pool(name="p3_ps", bufs=2, space="PSUM") as ps4pool, \
             tc_ctx.tile_pool(name="p3_ag", bufs=3) as agpool, \
             tc_ctx.tile_pool(name="p3_res", bufs=2) as respool:
            wo_sb = wopool.tile([128, N_HT, JC], BF16)
            nc.scalar.dma_start(out=wo_sb, in_=wot.ap())

            def emit_oproj(k):
                qc_k, cb = divmod(k, 2 * B)
                b_k, sub = divmod(cb, 2)
                t0 = b_k * S + qc_k * TC + sub * AGC
                ag_sb = agpool.tile([128, N_HT, AGC], BF16, tag="ag",
                                    name="ag_sb")
                ld = nc.sync.dma_start(
                    out=ag_sb,
                    in_=ag_outs[k].rearrange("(ht p) t -> p ht t", p=128),
                )
                if pace["inst"] is not None:
                    # Pace AllGather-output consumption behind real attention
                    # progress; the scheduler's collective latency estimate is
                    # optimistic and otherwise stalls the in-order PE stream.
                    bass._add_dep_helper(
                        ld.ins, pace["inst"], sync=True,
                        reason="oproj paced behind attention",
                    )
                for jt in range(JC // 128):
                    ps4 = ps4pool.tile([128, AGC], F32, name="ps4")
                    for h in range(N_HT):
                        nc.tensor.matmul(
                            ps4,
                            lhsT=wo_sb[:, h, jt * 128:(jt + 1) * 128],
                            rhs=ag_sb[:, h, :],
                            start=(h == 0),
                            stop=(h == N_HT - 1),
                        )
                    res = respool.tile([128, AGC], F32, name="res")
                    nc.scalar.copy(res, ps4)
                    nc.scalar.dma_start(
                        out=out_t[jt * 128:(jt + 1) * 128, t0:t0 + AGC],
                        in_=res,
                    )

            for k in pending:
                emit_oproj(k)


def _build_program():
    nc = bacc.Bacc("TRN2", target_bir_lowering=False, debug=False,
                   num_devices=N_CORES)
    xt = nc.declare_dram_parameter("xt", [128, N_CH, N_HT, TC], BF16,
                                   isOutput=False)
    wqkvt = nc.declare_dram_parameter("wqkvt", [128, HL + 2, N_HT, 128], BF16,
                                      isOutput=False)
    wot = nc.declare_dram_parameter("wot", [128, N_HT, JC], BF16,
                                    isOutput=False)
    ropes = nc.declare_dram_parameter("ropes", [128, N_CH, 4, TC], BF16,
                                      isOutput=False)
    out_t = nc.declare_dram_parameter("out_t", [JC, T], F32, isOutput=True)

    ag_ins = [nc.dram_tensor(f"ag_in{k}", [HL * D, AGC], BF16).ap()
              for k in range(N_AGCH)]
    ag_outs = [nc.dram_tensor(f"ag_out{k}", [N_HEADS * D, AGC], BF16,
                              addr_space="Shared").ap()
               for k in range(N_AGCH)]

    with tile.TileContext(nc) as tc_ctx:
        _emit(tc_ctx, xt, wqkvt, wot, ropes, out_t, ag_ins, ag_outs)
    nc.finalize()
    return nc


def _host_inputs(hidden_states, w_qkv, w_o):
    """Shard + transpose inputs for the 8 cores; returns in_maps."""
    X = np.asarray(hidden_states, dtype=np.float32).reshape(T, HID)
    xt = np.ascontiguousarray(
        X.reshape(N_CH, TC, N_HT, 128).transpose(3, 0, 2, 1)
    ).astype(ml_dtypes.bfloat16)

    # RoPE tables in [d, t] layout with rotate-half sign folded into sin and
    # the attention scale folded into the Q tables.
    inv_freq = 1.0 / (ROPE_BASE ** (np.arange(0, D, 2, dtype=np.float32) / D))
    pos = np.arange(S, dtype=np.float32)
    freqs = np.outer(pos, inv_freq)                      # (S, D/2)
    emb = np.concatenate([freqs, freqs], axis=-1)        # (S, D)
    cos = np.cos(emb).T.astype(np.float32)               # (D, S)
    sin = np.sin(emb).T.astype(np.float32)
    sgn = np.concatenate([-np.ones(D // 2), np.ones(D // 2)]).astype(np.float32)
    sins = sgn[:, None] * sin
    cos_t = np.tile(cos, (1, B))                         # (D, T)
    sins_t = np.tile(sins, (1, B))
    scale = np.float32(D ** -0.5)
    ropes = np.stack([cos_t * scale, sins_t * scale, cos_t, sins_t], axis=0)
    ropes = np.ascontiguousarray(
        ropes.reshape(4, 128, N_CH, TC).transpose(1, 2, 0, 3)
    ).astype(ml_dtypes.bfloat16)

    w_qkv = np.asarray(w_qkv, dtype=np.float32)
    w_o = np.asarray(w_o, dtype=np.float32)
    q_sz = N_HEADS * D
    kv_sz = N_KV_HEADS * D
    in_maps = []
    for c in range(N_CORES):
        qr = w_qkv[c * HL * D:(c + 1) * HL * D]
        kr = w_qkv[q_sz + c * D:q_sz + (c + 1) * D]
        vr = w_qkv[q_sz + kv_sz + c * D:q_sz + kv_sz + (c + 1) * D]
        w_shard = np.concatenate([qr, kr, vr], axis=0)           # (768, HID)
        wqkvt_c = np.ascontiguousarray(
            w_shard.reshape(HL + 2, 128, N_HT, 128).transpose(3, 0, 2, 1)
        ).astype(ml_dtypes.bfloat16)
        wot_c = np.ascontiguousarray(
            w_o[c * JC:(c + 1) * JC, :].reshape(JC, N_HT, 128).transpose(2, 1, 0)
        ).astype(ml_dtypes.bfloat16)
        in_maps.append({
            "xt": xt, "wqkvt": wqkvt_c, "wot": wot_c, "ropes": ropes,
        })
    return in_maps


def _run(hidden_states, w_qkv, w_o, trace=False, tmpdir=None):
    in_maps = _host_inputs(hidden_states, w_qkv, w_o)
    nc = _build_program()
    res = run_bass_kernel_spmd(nc, in_maps, list(range(N_CORES)),
                               trace=trace, tmpdir=tmpdir)
    out_T = np.concatenate(
        [np.asarray(res.results[c]["out_t"]) for c in range(N_CORES)], axis=0
    )                                                     # (HID j, T)
    out = np.ascontiguousarray(out_T.T).reshape(B, S, HID).astype(np.float32)
    return out, res


def kernel(hidden_states, w_qkv, w_o):
    out, _ = _run(hidden_states, w_qkv, w_o, trace=False)
    return out
